# revision 18
# baseline (speedup 1.0000x reference)
"""DAM module (conv3x3+BN+ReLU -> CAM + PAM attention) on 8 trn2 NeuronCores.

Sharding: core c -> (sample b=c//2, spatial-half h=c%2). Each core computes
the full conv for its sample (bf16 matmuls, fp32 PSUM accum), then CAM and
PAM attention restricted to its half of the output columns. The spatial
order is per-core permuted on the host (own half first) so the compiled
program is identical on every core.

v2 schedule notes (on top of the v1 software-pipelined kernel):
  * Input DMA ordered so conv pass 0 can start as soon as the first conv
    weights + own-half x pieces land (v1 queued all of x first and the PE
    sat idle for ~16us waiting on the conv weights).
  * The tanh ACT table is preloaded at kernel start; v1 paid a 1.3us
    ACT_TABLE_LOAD right in the post-collective critical chain, which
    contributed to a >3.4us PE idle that latched the HAM clock gate at
    1.2GHz for the rest of the kernel (~60% of PE work at half clock).
  * The BN-chain latency is bridged with short dummy-matmul groups gated
    on successive chain stages (star8 -> rstd -> bcoef) so the PE never
    idles long enough to re-throttle.
  * tmp1 path removed: the 3*feat residual is taken from the bf16 feat
    tile directly (one DVE mul per block in the steady loop) instead of
    a separate fp32 relu path; this unclogs the DVE FIFO right when the
    PE needs the q/k PSUM drains.
  * q and k projections merged into one matmul per block (stationary
    [128,48] = qwT | 0 | kwT), output q rows 0-15 / k rows 32-47.
  * kq/qp PSUM moved to the double-buffered pacc pool so consecutive
    blocks don't serialize through a single PSUM slot.

PSUM budget (8 banks): p2k 2x[128,1024] (conv passes, transposes, energy
double-buffer) + pacc 2x[128,512] (dummies, kq projections, apply
accumulators) + pcam 1x[128,512] (CAM energy, then CAM out) + pmisc
1x[128,512] (attnT transpose).
"""

import sys

for _p in ("/opt/trn_rl_repo",):
    if _p not in sys.path:
        sys.path.insert(0, _p)

from contextlib import ExitStack

import numpy as np
import ml_dtypes

import concourse.bass as bass
import concourse.bacc as bacc
import concourse.tile as tile
from concourse import mybir, masks
from concourse.bass_utils import run_bass_kernel_spmd

BF16NP = ml_dtypes.bfloat16
FP32 = mybir.dt.float32
BF16 = mybir.dt.bfloat16

B, CIN, COUT, H, W = 4, 256, 128, 64, 64
N = H * W          # 4096
NH = N // 2        # 2048 (one spatial half)
CQK = 16
EPS = 1e-5
NCORES = 8
PADH, PADW = 34, 66          # 32+2 halo rows, 64+2 halo cols
XPF = PADH * PADW            # 2244
NBLK = NH // 512             # 4 blocks of 512 per half
NCH = N // 128               # 32 chunks of 128 spatial positions
PRE_MMS = 6                  # bf16 warm-keepers bridging conv-end -> AG
LAD0_MMS = 2                 # fp32 warm-keepers gated on star8
LAD1_MMS = 1                 # gated on rstd
LAD2_MMS = 1                 # gated on bcoef


def _build_body(ctx: ExitStack, tc: tile.TileContext, io: dict, ga: float, gp: float):
    nc = tc.nc
    AX = mybir.AxisListType.X
    OP = mybir.AluOpType
    AF = mybir.ActivationFunctionType

    sb = ctx.enter_context(tc.tile_pool(name="sb", bufs=1))
    work = ctx.enter_context(tc.tile_pool(name="work", bufs=1))
    dram = ctx.enter_context(tc.tile_pool(name="dram", bufs=1, space="DRAM"))
    p2k = ctx.enter_context(tc.tile_pool(name="p2k", bufs=2, space="PSUM"))
    pacc = ctx.enter_context(tc.tile_pool(name="pacc", bufs=2, space="PSUM"))
    pcam = ctx.enter_context(tc.tile_pool(name="pcam", bufs=1, space="PSUM"))
    pmisc = ctx.enter_context(tc.tile_pool(name="pmisc", bufs=1, space="PSUM"))

    # ---- tanh ACT table preload (keeps the 1.3us table load out of the
    # post-collective critical chain) ----
    warmz = sb.tile([128, 2], FP32, tag="warmz")
    nc.vector.memset(warmz[:], 0.0)
    warmt = sb.tile([128, 1], BF16, tag="warmt")
    nc.scalar.activation(out=warmt[:], in_=warmz[:, 0:1], func=AF.Tanh)

    # ---- input DMAs, ordered to unlock conv pass 0 ASAP ----
    cw_sb = sb.tile([128, 18 * 128], BF16, tag="cw")
    x_sb = []
    for i in range(4):
        xt = sb.tile([128, XPF], BF16, tag=f"xp{i}")
        x_sb.append(xt)
    HALF1 = 17 * PADW
    cuts = [0, 9 * PADW, HALF1, 26 * PADW, XPF]

    def xpiece(i, p):
        nc.sync.dma_start(out=x_sb[i][:, cuts[p]:cuts[p + 1]],
                          in_=io["xp"][i][:, cuts[p]:cuts[p + 1]])

    nc.sync.dma_start(out=cw_sb[:, 0:576], in_=io["cw"][:, 0:576])
    xpiece(0, 0)
    xpiece(0, 1)
    nc.sync.dma_start(out=cw_sb[:, 576:1152], in_=io["cw"][:, 576:1152])
    xpiece(0, 2)
    for p in range(3):
        xpiece(1, p)
    nc.sync.dma_start(out=cw_sb[:, 1152:1728], in_=io["cw"][:, 1152:1728])
    nc.sync.dma_start(out=cw_sb[:, 1728:2304], in_=io["cw"][:, 1728:2304])
    xpiece(0, 3)
    xpiece(1, 3)

    # ---- small constants / weights (issued on the vector queue so they
    # don't serialize the sync queue's x/cw stream) ----
    kvw_sb = sb.tile([128, 384], BF16, tag="kvw")
    nc.gpsimd.dma_start(out=kvw_sb[:], in_=io["kvw"])
    # q/k biases 8x-replicated across partitions (col 0 = q/8, col 1 = k)
    qkb_sb = sb.tile([128, 2], FP32, tag="qkb")
    nc.gpsimd.dma_start(out=qkb_sb[:], in_=io["qkb"])
    # bn gamma/beta (col 0 = gamma, col 1 = beta)
    bnp_sb = sb.tile([128, 2], FP32, tag="bnp")
    nc.gpsimd.dma_start(out=bnp_sb[:], in_=io["bnp"])
    # v bias broadcast across partitions (DMA partition-step-0 replication)
    vbb = sb.tile([128, 128], FP32, tag="vbb")
    vb_ap = io["vb"]
    nc.gpsimd.dma_start(
        out=vbb[:],
        in_=bass.AP(tensor=vb_ap.tensor, offset=vb_ap.offset, ap=[[0, 128], [1, 128]]),
    )
    ident = sb.tile([128, 128], BF16, tag="ident")
    masks.make_identity(nc, ident[:])

    # other-half x pieces stream in during conv passes 0-1, issued on the
    # scalar queue (idle until the BN sqrt) to unclog the sync queue
    for i in (2, 3):
        for p in range(4):
            nc.scalar.dma_start(out=x_sb[i][:, cuts[p]:cuts[p + 1]],
                                in_=io["xp"][i][:, cuts[p]:cuts[p + 1]])

    # ---- conv3x3: y[cout, n] in bf16; stats (DVE) for OWN half only ----
    yb = sb.tile([128, N], BF16, tag="yb")
    sums4 = sb.tile([128, 4], FP32, tag="sums4")
    sq4 = sb.tile([128, 4], FP32, tag="sq4")

    # 4 passes of 2 blocks each; weight-outer so each pass does 18
    # LDWEIGHTS and 36 back-to-back matmuls into a [128,1024] accumulator.
    for p in range(4):
        yp = p2k.tile([128, 1024], FP32, tag="big")
        m = 0
        for k in range(2):
            for di in range(3):
                for dj in range(3):
                    wi = 9 * k + di * 3 + dj
                    for r in range(2):
                        blk = 2 * p + r          # global 512-block index
                        s, j = blk // NBLK, blk % NBLK
                        xv = x_sb[s * 2 + k][:].rearrange(
                            "p (r w) -> p r w", w=PADW)
                        nc.tensor.matmul(
                            yp[:, r * 512:(r + 1) * 512],
                            cw_sb[:, wi * 128:(wi + 1) * 128],
                            xv[:, 8 * j + di: 8 * j + di + 8, dj: dj + 64],
                            start=(m < 2),
                            stop=(m >= 34),
                            skip_group_check=True,
                        )
                        m += 1
        for r in range(2):
            t = 2 * p + r
            ypr = yp[:, r * 512:(r + 1) * 512]
            if p < 2:
                # own-half stats only; sum on DVE, sum-of-squares on ACT
                nc.vector.reduce_sum(out=sums4[:, t: t + 1], in_=ypr, axis=AX)
                scr = work.tile([128, 512], BF16, tag="scr", bufs=2)
                nc.scalar.activation(out=scr[:], in_=ypr, func=AF.Square,
                                     accum_out=sq4[:, t: t + 1])
            nc.vector.tensor_copy(out=yb[:, t * 512:(t + 1) * 512], in_=ypr)

        if p == 1:
            # ---- BN stats AllGather across all 8 cores (own halves) ----
            st = sb.tile([128, 2], FP32, tag="st")
            nc.vector.reduce_sum(out=st[:, 0:1], in_=sums4[:], axis=AX)
            nc.vector.reduce_sum(out=st[:, 1:2], in_=sq4[:], axis=AX)
            cc_in = dram.tile([128, 2], FP32, tag="ccin")
            cc_out = dram.tile([1024, 2], FP32, tag="ccout")
            nc.gpsimd.dma_start(out=cc_in[:], in_=st[:])
            nc.gpsimd.collective_compute(
                "AllGather", OP.bypass, ins=[cc_in.opt()], outs=[cc_out.opt()],
                replica_groups=[[0, 1, 2, 3, 4, 5, 6, 7]],
            )

    # ---- gather the 8 contributions (single strided DMA) and reduce ----
    star8 = sb.tile([128, 16], FP32, tag="star8")
    cco = cc_out[:]
    nc.sync.dma_start(
        out=star8[:],
        in_=bass.AP(tensor=cco.tensor, offset=cco.offset,
                    ap=[[2, 128], [256, 8], [1, 2]]),
    )
    lad0 = pacc.tile([128, 512], FP32, tag="acc")
    for i in range(3):
        nc.tensor.matmul(lad0[0:16, 0:128], star8[:], vbb[:],
                         start=(i == 0), stop=(i == 2))

    star = sb.tile([128, 2], FP32, tag="star")
    nc.vector.reduce_sum(
        out=star[:].rearrange("p (t o) -> p t o", o=1),
        in_=star8[:].rearrange("p (i t) -> p t i", t=2),
        axis=AX,
    )

    # ---- BN coefficients (shortest serial chain):
    # feat = relu(a*y - nb) with a = gamma/std, nb = mean*a - beta ----
    inv_n = 1.0 / float(B * N)
    star_n = sb.tile([128, 2], FP32, tag="star_n")
    nc.vector.tensor_scalar_mul(out=star_n[:], in0=star[:], scalar1=inv_n)
    mean = star_n[:, 0:1]
    var = sb.tile([128, 1], FP32, tag="var")
    mean2 = sb.tile([128, 1], FP32, tag="mean2")
    nc.vector.tensor_mul(out=mean2[:], in0=mean, in1=mean)
    nc.vector.tensor_sub(out=var[:], in0=star_n[:, 1:2], in1=mean2[:])
    eps_sb = sb.tile([128, 1], FP32, tag="eps")
    nc.vector.memset(eps_sb[:], EPS)
    std = sb.tile([128, 1], FP32, tag="std")
    nc.scalar.activation(out=std[:], in_=var[:], func=AF.Sqrt, bias=eps_sb[:])
    rstd = sb.tile([128, 1], FP32, tag="rstd")
    nc.vector.reciprocal(out=rstd[:], in_=std[:])
    # re-assert the tanh ACT table right after the sqrt (the only other
    # table user); the reload overlaps the BN chain + ladder
    nc.scalar.activation(out=warmt[:], in_=warmz[:, 0:1], func=AF.Tanh)

    lad1 = pacc.tile([128, 512], FP32, tag="acc")
    for i in range(2):
        nc.tensor.matmul(lad1[0:1, 0:128], rstd[:], vbb[:],
                         start=(i == 0), stop=(i == 1))

    acoef = sb.tile([128, 1], FP32, tag="acoef")
    nc.vector.tensor_mul(out=acoef[:], in0=bnp_sb[:, 0:1], in1=rstd[:])
    nbcoef = sb.tile([128, 1], FP32, tag="nbcoef")
    nc.vector.scalar_tensor_tensor(
        out=nbcoef[:], in0=mean, scalar=acoef[:], in1=bnp_sb[:, 1:2],
        op0=OP.mult, op1=OP.subtract)
    bcoef = sb.tile([128, 1], FP32, tag="bcoef")
    nc.vector.tensor_scalar_mul(out=bcoef[:], in0=nbcoef[:], scalar1=-1.0)
    nbcoef_bc = bass.AP(tensor=nbcoef[:].tensor, offset=nbcoef[:].offset,
                        ap=[nbcoef[:].ap[0], [0, 512]])

    lad2 = pacc.tile([128, 512], FP32, tag="acc")
    for i in range(2):
        nc.tensor.matmul(lad2[0:1, 0:128], bcoef[:], vbb[:],
                         start=(i == 0), stop=(i == 1))

    # ---- feat = relu(a*y + b): own half on ACT (unblocks the first
    # energies fast), other half on DVE in parallel ----
    feat = sb.tile([128, N], BF16, tag="feat")
    q_sb = sb.tile([128, NH], BF16, tag="q")
    k_sb = sb.tile([128, N], BF16, tag="k")
    FP8 = mybir.dt.float8e4
    ft = sb.tile([128, N], BF16, tag="ft")
    vt = sb.tile([128, N], FP8, tag="vt")
    at_a = sb.tile([128, 16 * 1024], FP8, tag="at0")
    at_b = sb.tile([128, 16 * 1024], FP8, tag="at1")
    at_buf = [at_a, at_b]
    attnT = sb.tile([128, 128], BF16, tag="attnT")
    en_sb = sb.tile([128, 128], FP32, tag="en_sb")
    out_sb = sb.tile([128, NH], FP32, tag="osb")

    def emit_energy(j, hg):
        # k is 8x row-replicated (q scaled by 1/8 host-side), so each
        # chunk energy is a full 128-contraction matmul: full PE array
        # utilization (keeps the activity clock-gate at high clock) and
        # FWL-eligible weight loads. tanh drains the tile straight from
        # PSUM into the fp8 at buffer.
        ep = p2k.tile([128, 1024], FP32, tag="big")
        for r in range(2):
            c = 2 * hg + r
            nc.tensor.matmul(
                ep[:, r * 512:(r + 1) * 512],
                k_sb[:, c * 128:(c + 1) * 128],
                q_sb[:, j * 512:(j + 1) * 512],
                start=True, stop=True, skip_group_check=True,
            )
        nc.scalar.activation(
            out=at_buf[j % 2][:, hg * 1024:(hg + 1) * 1024],
            in_=ep[:], func=AF.Tanh)

    def emit_tp(bch):
        # transpose 4 chunks (ft) + v transform for the same 4 chunks
        tp = p2k.tile([128, 1024], FP32, tag="big")
        for u in range(4):
            t = 4 * bch + u
            nc.tensor.matmul(
                tp[:, u * 128:(u + 1) * 128],
                feat[:, t * 128:(t + 1) * 128],
                ident[:],
                start=True, stop=True, skip_group_check=True,
            )
        for u in range(4):
            t = 4 * bch + u
            nc.tensor.matmul(
                tp[:, (4 + u) * 128:(5 + u) * 128],
                feat[:, t * 128:(t + 1) * 128],
                kvw_sb[:, 256:384],
                start=True, stop=True, skip_group_check=True,
            )
        nc.vector.tensor_copy(out=ft[:, bch * 512:(bch + 1) * 512],
                              in_=tp[:, 0:512])
        nc.vector.tensor_add(
            out=vt[:, bch * 512:(bch + 1) * 512], in0=tp[:, 512:1024],
            in1=bass.AP(tensor=vbb[:].tensor, offset=vbb[:].offset,
                        ap=[vbb[:].ap[0], [0, 4], [1, 128]]))

    for j in range(8):
        fj = feat[:, j * 512:(j + 1) * 512]
        if j < NBLK:
            # own half on ACT: runs before the tanh stream exists, so the
            # first energies are unblocked fast
            nc.scalar.activation(
                out=fj,
                in_=yb[:, j * 512:(j + 1) * 512], func=AF.Relu,
                bias=bcoef[:], scale=acoef[:])
        else:
            # other half on DVE, in parallel with the ACT relu blocks
            fsc = work.tile([128, 512], BF16, tag="fsc", bufs=2)
            nc.vector.scalar_tensor_tensor(
                out=fsc[:],
                in0=yb[:, j * 512:(j + 1) * 512], scalar=acoef[:],
                in1=nbcoef_bc, op0=OP.mult, op1=OP.subtract)
            nc.vector.tensor_scalar_max(out=fj, in0=fsc[:], scalar1=0.0)
        # k/q projections with 8x row-replicated weights: full-width
        # matmuls and a single full-width DVE drain each
        kp = pacc.tile([128, 512], FP32, tag="acc")
        nc.tensor.matmul(kp[:], kvw_sb[:, 0:128], fj,
                         start=True, stop=True, skip_group_check=True)
        nc.vector.tensor_scalar_add(
            out=k_sb[:, j * 512:(j + 1) * 512], in0=kp[:],
            scalar1=qkb_sb[:, 1:2])
        if j < NBLK:
            qp = pacc.tile([128, 512], FP32, tag="acc")
            nc.tensor.matmul(qp[:], kvw_sb[:, 128:256], fj,
                             start=True, stop=True, skip_group_check=True)
            nc.vector.tensor_scalar_add(
                out=q_sb[:, j * 512:(j + 1) * 512], in0=qp[:],
                scalar1=qkb_sb[:, 0:1])
        # block-0 energies + transposes chase the feat/projection blocks;
        # from iter 2 on, block-1 energies are pulled in too so the tanh
        # stream never starves while DVE paces the k/q drains
        emit_energy(0, 2 * j)
        emit_energy(0, 2 * j + 1)
        if j >= 2:
            emit_energy(1, 2 * (j - 2))
            emit_energy(1, 2 * (j - 2) + 1)
        if j < NBLK:
            emit_tp(j)

    # Steady: energies of j+1 + one fp8-DoubleRow apply of j per group
    # (apply g consumes exactly tanh group g of the previous block); CAM
    # energy accumulation rides in steady-0.
    en_ps = pcam.tile([128, 512], FP32, tag="cam")
    for j in range(NBLK):
        ops = pacc.tile([128, 512], FP32, tag="acc")
        vtr = vt[:].rearrange("p (t c) -> p t c", c=128)
        atr = at_buf[j % 2][:].rearrange("p (t m) -> p t m", m=512)
        for hg in range(16):
            if j == 0 and hg in (0, 2, 4, 6):
                # other-half transposes (consumed from group 2*bch on)
                # chase their steady-0 consumers, keeping the j-loop DVE
                # free to pace the k/q drains
                emit_tp(4 + hg // 2)
            if j + 1 < NBLK and (j > 0 or hg >= 12):
                emit_energy(j + 1, hg)
            if j == 0:
                for u in range(2):
                    t = 2 * hg + u
                    nc.tensor.matmul(
                        en_ps[:, 0:128],
                        ft[:, t * 128:(t + 1) * 128],
                        ft[:, t * 128:(t + 1) * 128],
                        start=(t == 0), stop=(t == NCH - 1),
                        skip_group_check=True,
                    )
            nc.tensor.matmul(
                ops[:],
                vtr[:, 2 * hg:2 * hg + 2, :],
                atr[:, 2 * hg:2 * hg + 2, :],
                start=(hg == 0), stop=(hg == 15), skip_group_check=True,
                perf_mode=mybir.MatmulPerfMode.DoubleRow,
            )
        if j == 0:
            # CAM attention map: attn = tanh(max(en) - en), then transpose
            nc.vector.tensor_copy(out=en_sb[:], in_=en_ps[:, 0:128])
            mx = sb.tile([128, 1], FP32, tag="mx")
            nc.vector.reduce_max(out=mx[:], in_=en_sb[:], axis=AX)
            en_new = sb.tile([128, 128], FP32, tag="en_new")
            nc.vector.tensor_scalar(
                out=en_new[:], in0=en_sb[:], scalar1=mx[:], scalar2=-1.0,
                op0=OP.subtract, op1=OP.mult,
            )
            attn = sb.tile([128, 128], BF16, tag="attn")
            nc.scalar.activation(out=attn[:], in_=en_new[:], func=AF.Tanh)
            atp = pmisc.tile([128, 512], FP32, tag="m")
            nc.tensor.matmul(atp[:, 0:128], attn[:], ident[:],
                             start=True, stop=True, skip_group_check=True)
            nc.vector.tensor_copy(out=attnT[:], in_=atp[:, 0:128])
        # CAM term for block j, then the gamma-weighted accumulation + DMA
        cps = pcam.tile([128, 512], FP32, tag="cam")
        nc.tensor.matmul(cps[:], attnT[:], feat[:, j * 512:(j + 1) * 512],
                         start=True, stop=True)
        # residual: out = 3*feat + gp*pam + ga*cam
        nc.vector.tensor_scalar_mul(
            out=out_sb[:, j * 512:(j + 1) * 512],
            in0=feat[:, j * 512:(j + 1) * 512], scalar1=3.0)
        nc.vector.scalar_tensor_tensor(
            out=out_sb[:, j * 512:(j + 1) * 512],
            in0=ops[:], scalar=gp, in1=out_sb[:, j * 512:(j + 1) * 512],
            op0=OP.mult, op1=OP.add)
        nc.vector.scalar_tensor_tensor(
            out=out_sb[:, j * 512:(j + 1) * 512],
            in0=cps[:], scalar=ga, in1=out_sb[:, j * 512:(j + 1) * 512],
            op0=OP.mult, op1=OP.add)
        nc.sync.dma_start(out=io["out"][:, j * 512:(j + 1) * 512],
                          in_=out_sb[:, j * 512:(j + 1) * 512])


def build_nc(ga: float, gp: float):
    nc = bacc.Bacc("TRN2", target_bir_lowering=False, debug=False,
                   num_devices=NCORES)
    io = {
        "xp": nc.dram_tensor("xp", [4, 128, XPF], BF16, kind="ExternalInput").ap(),
        "cw": nc.dram_tensor("cw", [128, 18 * 128], BF16, kind="ExternalInput").ap(),
        "kvw": nc.dram_tensor("kvw", [128, 384], BF16, kind="ExternalInput").ap(),
        "qkb": nc.dram_tensor("qkb", [128, 2], FP32, kind="ExternalInput").ap(),
        "vb": nc.dram_tensor("vb", [1, 128], FP32, kind="ExternalInput").ap(),
        "bnp": nc.dram_tensor("bnp", [128, 2], FP32, kind="ExternalInput").ap(),
        "out": nc.dram_tensor("out", [128, NH], FP32, kind="ExternalOutput").ap(),
    }
    with tile.TileContext(nc) as tc, ExitStack() as ctx:
        _build_body(ctx, tc, io, ga, gp)
    nc.compile()
    return nc


def make_in_maps(x, conv_w, bn_gamma, bn_beta, q_w, q_b, k_w, k_b, v_w, v_b):
    x = np.asarray(x, np.float32)
    conv_w = np.asarray(conv_w, np.float32)

    xpad = np.zeros((B, CIN, H + 2, W + 2), np.float32)
    xpad[:, :, 1:H + 1, 1:W + 1] = x

    cw = np.empty((128, 18 * 128), np.float32)
    for di in range(3):
        for dj in range(3):
            for k in range(2):
                wi = 9 * k + di * 3 + dj
                cw[:, wi * 128:(wi + 1) * 128] = conv_w[:, k * 128:(k + 1) * 128, di, dj].T
    kvw = np.zeros((128, 384), np.float32)
    for g in range(8):
        kvw[:, g * CQK:(g + 1) * CQK] = np.asarray(k_w, np.float32).T
        kvw[:, 128 + g * CQK:128 + (g + 1) * CQK] = np.asarray(q_w, np.float32).T / 8.0
    kvw[:, 256:384] = np.asarray(v_w, np.float32).T
    qkb = np.stack([np.tile(np.asarray(q_b, np.float32) / 8.0, 8),
                    np.tile(np.asarray(k_b, np.float32), 8)], axis=1)
    bnp = np.stack([np.asarray(bn_gamma, np.float32), np.asarray(bn_beta, np.float32)], axis=1)
    shared = {
        "cw": cw.astype(BF16NP),
        "kvw": kvw.astype(BF16NP),
        "qkb": np.ascontiguousarray(qkb),
        "vb": np.asarray(v_b, np.float32).reshape(1, 128),
        "bnp": np.ascontiguousarray(bnp),
    }

    in_maps = []
    for c in range(NCORES):
        b, h = c // 2, c % 2
        xp = np.empty((4, 128, XPF), np.float32)
        for s, half in enumerate((h, 1 - h)):
            blk = xpad[b, :, 32 * half:32 * half + PADH, :]  # [256, 34, 66]
            for k in range(2):
                xp[s * 2 + k] = blk[k * 128:(k + 1) * 128].reshape(128, XPF)
        m = dict(shared)
        m["xp"] = xp.astype(BF16NP)
        in_maps.append(m)
    return in_maps


_NC_CACHE: dict = {}


def kernel(x, conv_w, bn_gamma, bn_beta, q_w, q_b, k_w, k_b, v_w, v_b,
           gamma_ca, gamma_pa):
    ga = float(np.asarray(gamma_ca).reshape(-1)[0])
    gp = float(np.asarray(gamma_pa).reshape(-1)[0])
    key = (ga, gp)
    if key not in _NC_CACHE:
        _NC_CACHE[key] = build_nc(ga, gp)
    nc = _NC_CACHE[key]

    in_maps = make_in_maps(x, conv_w, bn_gamma, bn_beta,
                           q_w, q_b, k_w, k_b, v_w, v_b)
    res = run_bass_kernel_spmd(nc, in_maps, core_ids=list(range(NCORES)))

    out = np.empty((B, COUT, H, W), np.float32)
    for c in range(NCORES):
        b, h = c // 2, c % 2
        out[b, :, 32 * h:32 * h + 32, :] = \
            res.results[c]["out"].reshape(COUT, 32, W)
    return out
